# revision 18
# baseline (speedup 1.0000x reference)
"""Sequence-parallel attention kernel for 8 TRN2 NeuronCores (bf16 pipeline).

Reference computation (all fp32):
    Q = x @ Wq.T ; K = x @ Wk.T ; V = x @ Wv.T
    S = Q @ K.T / sqrt(1024)
    out = softmax(S, axis=-1) @ V

Math restructure (identical result, 1/8 of the reference FLOPs per core,
weight-only products folded on the host):
    At = (Wq.T @ Wk)              host fold   [d, d]   (lhsT of A = Wk.T Wq)
    Pt[b, q] = sum_j At[j, b] xqt[j, q]       [d, 512]  per-core query block
    St[k, q] = sum_b xt[b, k] Pt[b, q]        (scores transposed, streamed)
    E  = exp(St / 32)                         bf16 tiles, SBUF-resident
    denom[q] = sum_k E[k, q]                  (ones-vector matmul chain)
    Ut[c, q] = sum_k x[k, c] E[k, q]          (PSUM chains spanning all k,
                                               c split into two sweeps)
    out[q, dv] = (sum_c Ut[c, q] WvT[c, dv]) / denom[q]

All matmul operands are bfloat16 (full PE rate, half the DMA/SBUF of f32r;
overall rel err ~3e-3 vs fp32 reference).  E tiles live in SBUF so the Ut
accumulation runs as 8 long PSUM chains with no per-superchunk DVE adds.
Ut PSUM banks are memset once and accumulated with start=False to avoid
the hardware behaviour where start_tensor_calc zeroes the full bank row.
"""

import sys

sys.path.insert(0, "/opt/trn_rl_repo")

import numpy as np
import ml_dtypes

import concourse.tile as tile
from concourse import bacc, mybir
from concourse.bass_utils import run_bass_kernel_spmd

F32 = mybir.dt.float32
BF16 = mybir.dt.bfloat16

S = 4096          # sequence length
D = 1024          # d_in == d_out
P = 128           # partitions
NCORES = 8
R = S // NCORES   # query rows per core (512)
KSC = 512         # key super-chunk (DMA prefetch granularity)
NSC = S // KSC    # 8 super-chunks
NKB = S // P      # 32 key blocks
DC = D // P       # 8 chunks of the model dim
QC = R // P       # 4 query chunks per core
SCALE = 1.0 / np.sqrt(np.float32(D))
BF = ml_dtypes.bfloat16


def build_program():
    nc = bacc.Bacc("TRN2", target_bir_lowering=False, debug=False,
                   num_devices=NCORES)

    at_d = nc.dram_tensor("at", [D, D], BF16, kind="ExternalInput").ap()
    xqt_d = nc.dram_tensor("xqt", [D // 2, 2 * R], BF16, kind="ExternalInput").ap()
    xtb_d = nc.dram_tensor("xtb", [NSC * (DC // 2) * P, 2 * KSC], BF16, kind="ExternalInput").ap()
    xb_d = nc.dram_tensor("xb", [S, D], BF16, kind="ExternalInput").ap()
    wvt_d = nc.dram_tensor("wvt", [D, D], BF16, kind="ExternalInput").ap()
    out_d = nc.dram_tensor("out", [R, D], BF16, kind="ExternalOutput").ap()
    # 2-D ExternalOutput: internal DRAM tensors (and 1-D I/O tensors) fail
    # to load under the axon/PJRT path.
    dscratch = nc.dram_tensor("dscratch", [1, R], F32, kind="ExternalOutput").ap()

    with tile.TileContext(nc) as tc:
        _emit(tc, at_d, xqt_d, xtb_d, xb_d, wvt_d, out_d, dscratch)

    nc.compile()
    return nc


def _emit(tc, at_d, xqt_d, xtb_d, xb_d, wvt_d, out_d, dscratch):
    nc = tc.nc
    from contextlib import ExitStack

    with ExitStack() as ctx:
        const = ctx.enter_context(tc.tile_pool(name="const", bufs=1))
        ps_mm = ctx.enter_context(tc.tile_pool(name="ps_mm", bufs=3, space="PSUM"))
        ps_dn = ctx.enter_context(tc.tile_pool(name="ps_dn", bufs=1, space="PSUM"))
        ps_ut = ctx.enter_context(tc.tile_pool(name="ps_ut", bufs=4, space="PSUM"))
        big = ctx.enter_context(tc.tile_pool(name="big", bufs=1))
        xts_pool = ctx.enter_context(tc.tile_pool(name="xts", bufs=8))
        dn_pool = ctx.enter_context(tc.tile_pool(name="dn", bufs=1))

        ones_f = const.tile([P, 1], F32)
        nc.vector.memset(ones_f, 1.0)
        ones_b = const.tile([P, 1], BF16)
        nc.vector.tensor_copy(ones_b, ones_f)

        e_sb = big.tile([P, NKB, R], BF16)    # E[k, q]   32 KB/part
        xn_sb = big.tile([P, NKB, D], BF16)   # x[k, c]   64 KB/part
        pt_sb = big.tile([P, DC, R], BF16)    # Pt[b, q]   8 KB/part
        wvt_sb = big.tile([P, DC, D], BF16)   # WvT[c,dv] 16 KB/part
        ut_b = big.tile([P, DC, R], BF16)     # Ut[c, q]   8 KB/part

        def prefetch(sc):
            """xt columns (St stationary) + x rows (Ut stationary) of sc."""
            xts = []
            for bp in range(DC // 2):
                t = xts_pool.tile([P, 2, KSC], BF16, tag="xts")
                nc.sync.dma_start(
                    out=t,
                    in_=xtb_d[(sc * (DC // 2) + bp) * P:
                              (sc * (DC // 2) + bp + 1) * P, :])
                xts.append(t)
            for kc in range(KSC // P):
                kb = sc * (KSC // P) + kc
                nc.sync.dma_start(out=xn_sb[:, kb, :],
                                  in_=xb_d[kb * P:(kb + 1) * P, :])
            return xts

        # ---- Phase Pt: Pt = A @ xqt  (bb-outer, 2 rotating banks) ----
        # At is loaded as column-block tiles so chain bb only waits on its
        # own 256 KB slice; prefetch DMAs are emitted after the Pt matmuls
        # so the At/xqt loads get the full DMA bandwidth at startup.
        with tc.tile_pool(name="early", bufs=1) as early:
            # chain bb=0 needs xqt[0] + at_col[0] only: land those first,
            # then the remaining xqt pairs (chain 0 consumes them in order),
            # then the other at column tiles (one per later chain).
            xqt_pairs = [early.tile([P, 2, R], BF16, tag=f"xqt{jp}",
                                    name=f"xqt{jp}")
                         for jp in range(DC // 2)]
            at_cols = [early.tile([P, DC, P], BF16, tag=f"at{bb}",
                                  name=f"at{bb}")
                       for bb in range(DC)]
            nc.sync.dma_start(out=xqt_pairs[0], in_=xqt_d[0:P, :])
            nc.sync.dma_start(out=at_cols[0], in_=at_d[0:P, :])
            for jp in range(1, DC // 2):
                nc.sync.dma_start(out=xqt_pairs[jp],
                                  in_=xqt_d[jp * P:(jp + 1) * P, :])

            # Interleave the remaining at-column loads with superchunk-0
            # tiles so the Pt chains stay PE-bound while sc0 streams in.
            sc0_parts = []
            for bp in range(DC // 2):
                sc0_parts.append(("xts", bp))
            for kc in range(KSC // P):
                sc0_parts.append(("xn", kc))
            xts0 = []
            for bb in range(1, DC):
                nc.sync.dma_start(out=at_cols[bb],
                                  in_=at_d[bb * P:(bb + 1) * P, :])
                kind, idx = sc0_parts[bb - 1]
                if kind == "xts":
                    t = xts_pool.tile([P, 2, KSC], BF16, tag="xts",
                                      name=f"xts0_{idx}")
                    nc.sync.dma_start(out=t, in_=xtb_d[idx * P:(idx + 1) * P, :])
                    xts0.append(t)
                else:
                    nc.sync.dma_start(out=xn_sb[:, idx, :],
                                      in_=xb_d[idx * P:(idx + 1) * P, :])
            kind, idx = sc0_parts[-1]
            nc.sync.dma_start(out=xn_sb[:, idx, :],
                              in_=xb_d[idx * P:(idx + 1) * P, :])

            pf = {0: xts0, 1: prefetch(1)}

            # Ut accumulator banks: memset once, chains use start=False.
            ut_ps = [ps_ut.tile([P, R], F32, tag="ut", name=f"ut_ps{i}")
                     for i in range(4)]
            for t in ut_ps:
                nc.vector.memset(t, 0.0)

            for bb in range(DC):
                ps = ps_mm.tile([P, R], F32, tag="mm")
                for jc in range(DC):
                    nc.tensor.matmul(
                        ps,
                        at_cols[bb][:, jc, :],
                        xqt_pairs[jc // 2][:, jc % 2, :],
                        start=(jc == 0), stop=(jc == DC - 1),
                    )
                if bb % 2 == 0:
                    nc.vector.tensor_copy(pt_sb[:, bb, :], ps)
                else:
                    nc.scalar.copy(pt_sb[:, bb, :], ps)

        denom_ps = ps_dn.tile([1, R], F32)

        s1ccs = list(range(4))

        def lagged_mms(kb):
            """denom + Ut(cc 0..3) matmuls for key-block kb (interleaved
            into a later St chain so boundary ldws hide under St mms)."""
            yield lambda: nc.tensor.matmul(
                denom_ps, ones_b, e_sb[:, kb, :],
                start=(kb == 0), stop=(kb == NKB - 1),
            )
            for i, cc in enumerate(s1ccs):
                yield lambda i=i, cc=cc: nc.tensor.matmul(
                    ut_ps[i],
                    xn_sb[:, kb, cc * P:(cc + 1) * P],
                    e_sb[:, kb, :],
                    start=False, stop=(kb == NKB - 1),
                    skip_group_check=True,
                )

        # ---- Sweep 1: St -> exp -> E tiles; denom + Ut(cc 0..3) lag 2 kb ----
        # (two-block lag gives the ACT exp plenty of time so the denom/Ut
        # matmuls never stall on the activation semaphore)
        LAG = 2
        for sc in range(NSC):
            xts = pf.pop(sc)
            if sc + 2 < NSC:
                pf[sc + 2] = prefetch(sc + 2)
            for kc in range(KSC // P):
                kb = sc * (KSC // P) + kc
                ps = ps_mm.tile([P, R], F32, tag="mm")
                for bb in range(DC):
                    nc.tensor.matmul(
                        ps,
                        xts[bb // 2][:, bb % 2, kc * P:(kc + 1) * P],
                        pt_sb[:, bb, :],
                        start=(bb == 0), stop=(bb == DC - 1),
                    )
                nc.scalar.activation(e_sb[:, kb, :], ps,
                                     mybir.ActivationFunctionType.Exp,
                                     scale=float(SCALE))
                if kb >= LAG + 1 and kb % 2 == 1:
                    for f in lagged_mms(kb - LAG - 1):
                        f()
                    for f in lagged_mms(kb - LAG):
                        f()
        for kb in range(NKB - LAG, NKB):
            for f in lagged_mms(kb):
                f()

        # denom -> [q, 1] layout via DRAM round-trip (overlaps sweep 2)
        denom_sb = dn_pool.tile([1, R], F32, tag="dsb")
        nc.vector.tensor_copy(denom_sb, denom_ps)
        nc.sync.dma_start(out=dscratch, in_=denom_sb)
        dt_sb = dn_pool.tile([P, QC], F32, tag="dt")
        nc.sync.dma_start(out=dt_sb, in_=dscratch.rearrange("o (j p) -> (o p) j", p=P))
        recip = dn_pool.tile([P, QC], F32, tag="recip")
        nc.vector.reciprocal(recip, dt_sb)

        # wvt only needed by the out phase; load during sweep 2.
        for cw in range(DC):
            nc.sync.dma_start(out=wvt_sb[:, cw, :],
                              in_=wvt_d[cw * P:(cw + 1) * P, :])

        # ---- Sweep 2: Ut(cc 4..7), cc-outer so bank reuse pipelines ----
        # convert sweep-1 banks as sweep 2 proceeds
        def convert_ut(i, cc):
            if cc % 2 == 0:
                nc.vector.tensor_copy(ut_b[:, cc, :], ut_ps[i])
            else:
                nc.scalar.copy(ut_b[:, cc, :], ut_ps[i])

        for i, cc in enumerate(s1ccs):
            convert_ut(i, cc)

        for cc in range(4, DC):
            t = ps_ut.tile([P, R], F32, tag="ut", name=f"ut_ps{cc}")
            nc.vector.memset(t, 0.0)
            for kb in range(NKB):
                nc.tensor.matmul(
                    t,
                    xn_sb[:, kb, cc * P:(cc + 1) * P],
                    e_sb[:, kb, :],
                    start=False, stop=(kb == NKB - 1),
                    skip_group_check=True,
                )
            ut_ps.append(t)
            convert_ut(4 + (cc - 4), cc)

        # ---- out[q, dv] = (sum_c Ut[c,q] WvT[c,dv]) * recip[q] ----
        with tc.tile_pool(name="outp", bufs=2) as outp:
            for cq in range(QC):
                ot = outp.tile([P, D], BF16, tag="out")
                for nd in range(2):
                    ps = ps_mm.tile([P, R], F32, tag="mm")
                    for cc in range(DC):
                        nc.tensor.matmul(
                            ps,
                            ut_b[:, cc, cq * P:(cq + 1) * P],
                            wvt_sb[:, cc, nd * R:(nd + 1) * R],
                            start=(cc == 0), stop=(cc == DC - 1),
                        )
                    nc.vector.tensor_scalar_mul(
                        ot[:, nd * R:(nd + 1) * R], ps, recip[:, cq:cq + 1])
                nc.sync.dma_start(out=out_d[cq * P:(cq + 1) * P, :], in_=ot)


_CACHE = {}


def _get_program():
    if "nc" not in _CACHE:
        _CACHE["nc"] = build_program()
    return _CACHE["nc"]


def make_in_maps(x, W_query, W_key, W_value):
    x = np.ascontiguousarray(x, dtype=np.float32)
    wq = np.asarray(W_query, dtype=np.float32)
    wk = np.asarray(W_key, dtype=np.float32)
    wv = np.asarray(W_value, dtype=np.float32)
    at_f = wq.T @ wk                                      # A.T = lhsT of Wk.T @ Wq
    # at rows (bb, p) hold all jc-blocks contiguously: 2 KB DMA rows
    at = np.ascontiguousarray(
        at_f.reshape(DC, P, DC, P).transpose(2, 1, 0, 3).reshape(D, D)).astype(BF)
    xt = np.ascontiguousarray(x.T)
    # xt rows (sc, bp, p) hold the (i, k) pair-block contiguously
    xtb = np.ascontiguousarray(
        xt.reshape(DC // 2, 2, P, NSC, KSC).transpose(3, 0, 2, 1, 4)
        .reshape(NSC * (DC // 2) * P, 2 * KSC)).astype(BF)
    xb = x.astype(BF)
    wvt = np.ascontiguousarray(wv.T).astype(BF)
    maps = []
    for i in range(NCORES):
        xqt_f = xt[:, i * R:(i + 1) * R]
        xqt = np.ascontiguousarray(
            xqt_f.reshape(DC // 2, 2, P, R).transpose(0, 2, 1, 3)
            .reshape(D // 2, 2 * R)).astype(BF)
        maps.append({"at": at, "xqt": xqt, "xtb": xtb, "xb": xb, "wvt": wvt})
    return maps


def kernel(x, W_query, W_key, W_value):
    nc = _get_program()
    in_maps = make_in_maps(x, W_query, W_key, W_value)
    res = run_bass_kernel_spmd(nc, in_maps, core_ids=list(range(NCORES)))
    return np.concatenate(
        [res.results[i]["out"] for i in range(NCORES)], axis=0
    ).astype(np.float32)


# revision 19
# speedup vs baseline: 1.0058x; 1.0058x over previous
"""Sequence-parallel attention kernel for 8 TRN2 NeuronCores (bf16 pipeline).

Reference computation (all fp32):
    Q = x @ Wq.T ; K = x @ Wk.T ; V = x @ Wv.T
    S = Q @ K.T / sqrt(1024)
    out = softmax(S, axis=-1) @ V

Math restructure (identical result, 1/8 of the reference FLOPs per core,
weight-only products folded on the host):
    At = (Wq.T @ Wk)              host fold   [d, d]   (lhsT of A = Wk.T Wq)
    Pt[b, q] = sum_j At[j, b] xqt[j, q]       [d, 512]  per-core query block
    St[k, q] = sum_b xt[b, k] Pt[b, q]        (scores transposed, streamed)
    E  = exp(St / 32)                         bf16 tiles, SBUF-resident
    denom[q] = sum_k E[k, q]                  (ones-vector matmul chain)
    Ut[c, q] = sum_k x[k, c] E[k, q]          (PSUM chains spanning all k,
                                               c split into two sweeps)
    out[q, dv] = (sum_c Ut[c, q] WvT[c, dv]) / denom[q]

All matmul operands are bfloat16 (full PE rate, half the DMA/SBUF of f32r;
overall rel err ~3.5e-3 vs fp32 reference).  E tiles live in SBUF so the Ut
accumulation runs as 8 long PSUM chains with no per-superchunk DVE adds.
Ut PSUM banks are memset once and accumulated with start=False to avoid
the hardware behaviour where start_tensor_calc zeroes the full bank row.

Schedule/bandwidth notes (measured on HW):
  - fp8 DoubleRow is NOT faster in practice (1.0 cycle/output-row on HW,
    same MAC/cycle as bf16) and every matmul self-loads weights, so the
    whole kernel stays bf16.
  - DMA descriptors with >=2 KB contiguous rows run ~2x faster (~400 GB/s
    vs ~190), so at/xqt/xt are uploaded in pre-swizzled layouts that make
    every transfer 2 KB-row contiguous.
  - Prefetch DMAs must be enqueued close behind the startup loads; pushing
    them after the Pt matmuls dropped the core clock ~20% (DVFS).
  - The device writes bf16 output (halves the tail DMA); the host casts
    back to float32.
"""

import sys

sys.path.insert(0, "/opt/trn_rl_repo")

import numpy as np
import ml_dtypes

import concourse.tile as tile
from concourse import bacc, mybir
from concourse.bass_utils import run_bass_kernel_spmd

F32 = mybir.dt.float32
BF16 = mybir.dt.bfloat16

S = 4096          # sequence length
D = 1024          # d_in == d_out
P = 128           # partitions
NCORES = 8
R = S // NCORES   # query rows per core (512)
KSC = 512         # key super-chunk (DMA prefetch granularity)
NSC = S // KSC    # 8 super-chunks
NKB = S // P      # 32 key blocks
DC = D // P       # 8 chunks of the model dim
QC = R // P       # 4 query chunks per core
SCALE = 1.0 / np.sqrt(np.float32(D))
BF = ml_dtypes.bfloat16


def build_program():
    nc = bacc.Bacc("TRN2", target_bir_lowering=False, debug=False,
                   num_devices=NCORES)

    at_d = nc.dram_tensor("at", [D, D], BF16, kind="ExternalInput").ap()
    xqt_d = nc.dram_tensor("xqt", [D // 2, 2 * R], BF16, kind="ExternalInput").ap()
    xtb_d = nc.dram_tensor("xtb", [NSC * (DC // 2) * P, 2 * KSC], BF16, kind="ExternalInput").ap()
    xb_d = nc.dram_tensor("xb", [S, D], BF16, kind="ExternalInput").ap()
    wvt_d = nc.dram_tensor("wvt", [D, D], BF16, kind="ExternalInput").ap()
    out_d = nc.dram_tensor("out", [R, D], BF16, kind="ExternalOutput").ap()
    # 2-D ExternalOutput: internal DRAM tensors (and 1-D I/O tensors) fail
    # to load under the axon/PJRT path.
    dscratch = nc.dram_tensor("dscratch", [1, R], F32, kind="ExternalOutput").ap()

    with tile.TileContext(nc) as tc:
        _emit(tc, at_d, xqt_d, xtb_d, xb_d, wvt_d, out_d, dscratch)

    nc.compile()
    return nc


def _emit(tc, at_d, xqt_d, xtb_d, xb_d, wvt_d, out_d, dscratch):
    nc = tc.nc
    from contextlib import ExitStack

    with ExitStack() as ctx:
        const = ctx.enter_context(tc.tile_pool(name="const", bufs=1))
        ps_mm = ctx.enter_context(tc.tile_pool(name="ps_mm", bufs=3, space="PSUM"))
        ps_dn = ctx.enter_context(tc.tile_pool(name="ps_dn", bufs=1, space="PSUM"))
        ps_ut = ctx.enter_context(tc.tile_pool(name="ps_ut", bufs=4, space="PSUM"))
        big = ctx.enter_context(tc.tile_pool(name="big", bufs=1))
        xts_pool = ctx.enter_context(tc.tile_pool(name="xts", bufs=8))
        dn_pool = ctx.enter_context(tc.tile_pool(name="dn", bufs=1))

        ones_f = const.tile([P, 1], F32)
        nc.vector.memset(ones_f, 1.0)
        ones_b = const.tile([P, 1], BF16)
        nc.vector.tensor_copy(ones_b, ones_f)

        e_sb = big.tile([P, NKB, R], BF16)    # E[k, q]   32 KB/part
        xn_sb = big.tile([P, NKB, D], BF16)   # x[k, c]   64 KB/part
        pt_sb = big.tile([P, DC, R], BF16)    # Pt[b, q]   8 KB/part
        wvt_sb = big.tile([P, DC, D], BF16)   # WvT[c,dv] 16 KB/part
        ut_b = big.tile([P, DC, R], BF16)     # Ut[c, q]   8 KB/part

        def prefetch(sc):
            """xt columns (St stationary) + x rows (Ut stationary) of sc."""
            xts = []
            for bp in range(DC // 2):
                t = xts_pool.tile([P, 2, KSC], BF16, tag="xts")
                nc.sync.dma_start(
                    out=t,
                    in_=xtb_d[(sc * (DC // 2) + bp) * P:
                              (sc * (DC // 2) + bp + 1) * P, :])
                xts.append(t)
            for kc in range(KSC // P):
                kb = sc * (KSC // P) + kc
                nc.sync.dma_start(out=xn_sb[:, kb, :],
                                  in_=xb_d[kb * P:(kb + 1) * P, :])
            return xts

        # ---- Phase Pt: Pt = A @ xqt  (bb-outer, 2 rotating banks) ----
        # At is loaded as column-block tiles so chain bb only waits on its
        # own 256 KB slice; prefetch DMAs are emitted after the Pt matmuls
        # so the At/xqt loads get the full DMA bandwidth at startup.
        with tc.tile_pool(name="early", bufs=1) as early:
            # chain bb=0 needs xqt[0] + at_col[0] only: land those first,
            # then the remaining xqt pairs (chain 0 consumes them in order),
            # then the other at column tiles (one per later chain).
            xqt_pairs = [early.tile([P, 2, R], BF16, tag=f"xqt{jp}",
                                    name=f"xqt{jp}")
                         for jp in range(DC // 2)]
            at_cols = [early.tile([P, DC, P], BF16, tag=f"at{bb}",
                                  name=f"at{bb}")
                       for bb in range(DC)]
            nc.sync.dma_start(out=xqt_pairs[0], in_=xqt_d[0:P, :])
            nc.sync.dma_start(out=at_cols[0], in_=at_d[0:P, :])
            for jp in range(1, DC // 2):
                nc.sync.dma_start(out=xqt_pairs[jp],
                                  in_=xqt_d[jp * P:(jp + 1) * P, :])

            # Interleave the remaining at-column loads with superchunk-0
            # tiles so the Pt chains stay PE-bound while sc0 streams in.
            sc0_parts = []
            for bp in range(DC // 2):
                sc0_parts.append(("xts", bp))
            for kc in range(KSC // P):
                sc0_parts.append(("xn", kc))
            xts0 = []
            for bb in range(1, DC):
                nc.sync.dma_start(out=at_cols[bb],
                                  in_=at_d[bb * P:(bb + 1) * P, :])
                kind, idx = sc0_parts[bb - 1]
                if kind == "xts":
                    t = xts_pool.tile([P, 2, KSC], BF16, tag="xts",
                                      name=f"xts0_{idx}")
                    nc.sync.dma_start(out=t, in_=xtb_d[idx * P:(idx + 1) * P, :])
                    xts0.append(t)
                else:
                    nc.sync.dma_start(out=xn_sb[:, idx, :],
                                      in_=xb_d[idx * P:(idx + 1) * P, :])
            kind, idx = sc0_parts[-1]
            nc.sync.dma_start(out=xn_sb[:, idx, :],
                              in_=xb_d[idx * P:(idx + 1) * P, :])

            pf = {0: xts0, 1: prefetch(1)}

            # Ut accumulator banks: memset once, chains use start=False.
            ut_ps = [ps_ut.tile([P, R], F32, tag="ut", name=f"ut_ps{i}")
                     for i in range(4)]
            for t in ut_ps:
                nc.vector.memset(t, 0.0)

            for bb in range(DC):
                ps = ps_mm.tile([P, R], F32, tag="mm")
                for jc in range(DC):
                    nc.tensor.matmul(
                        ps,
                        at_cols[bb][:, jc, :],
                        xqt_pairs[jc // 2][:, jc % 2, :],
                        start=(jc == 0), stop=(jc == DC - 1),
                    )
                if bb % 2 == 0:
                    nc.vector.tensor_copy(pt_sb[:, bb, :], ps)
                else:
                    nc.scalar.copy(pt_sb[:, bb, :], ps)

        denom_ps = ps_dn.tile([1, R], F32)

        s1ccs = list(range(4))

        def lagged_mms(kb):
            """denom + Ut(cc 0..3) matmuls for key-block kb (interleaved
            into a later St chain so boundary ldws hide under St mms)."""
            yield lambda: nc.tensor.matmul(
                denom_ps, ones_b, e_sb[:, kb, :],
                start=(kb == 0), stop=(kb == NKB - 1),
            )
            for i, cc in enumerate(s1ccs):
                yield lambda i=i, cc=cc: nc.tensor.matmul(
                    ut_ps[i],
                    xn_sb[:, kb, cc * P:(cc + 1) * P],
                    e_sb[:, kb, :],
                    start=False, stop=(kb == NKB - 1),
                    skip_group_check=True,
                )

        # ---- Sweep 1: St -> exp -> E tiles; denom + Ut(cc 0..3) lag 2 kb ----
        # (two-block lag gives the ACT exp plenty of time so the denom/Ut
        # matmuls never stall on the activation semaphore)
        LAG = 2
        for sc in range(NSC):
            xts = pf.pop(sc)
            if sc + 2 < NSC:
                pf[sc + 2] = prefetch(sc + 2)
            for kc in range(KSC // P):
                kb = sc * (KSC // P) + kc
                ps = ps_mm.tile([P, R], F32, tag="mm")
                for bb in range(DC):
                    nc.tensor.matmul(
                        ps,
                        xts[bb // 2][:, bb % 2, kc * P:(kc + 1) * P],
                        pt_sb[:, bb, :],
                        start=(bb == 0), stop=(bb == DC - 1),
                    )
                nc.scalar.activation(e_sb[:, kb, :], ps,
                                     mybir.ActivationFunctionType.Exp,
                                     scale=float(SCALE))
                if kb >= LAG + 1 and kb % 2 == 1:
                    for f in lagged_mms(kb - LAG - 1):
                        f()
                    for f in lagged_mms(kb - LAG):
                        f()
        for kb in range(NKB - LAG, NKB):
            for f in lagged_mms(kb):
                f()

        # denom -> [q, 1] layout via DRAM round-trip (overlaps sweep 2)
        denom_sb = dn_pool.tile([1, R], F32, tag="dsb")
        nc.vector.tensor_copy(denom_sb, denom_ps)
        nc.sync.dma_start(out=dscratch, in_=denom_sb)
        dt_sb = dn_pool.tile([P, QC], F32, tag="dt")
        nc.sync.dma_start(out=dt_sb, in_=dscratch.rearrange("o (j p) -> (o p) j", p=P))
        recip = dn_pool.tile([P, QC], F32, tag="recip")
        nc.vector.reciprocal(recip, dt_sb)

        # wvt only needed by the out phase; load during sweep 2.
        for cw in range(DC):
            nc.sync.dma_start(out=wvt_sb[:, cw, :],
                              in_=wvt_d[cw * P:(cw + 1) * P, :])

        # ---- Sweep 2: Ut(cc 4..7), cc-outer so bank reuse pipelines ----
        # convert sweep-1 banks as sweep 2 proceeds
        def convert_ut(i, cc):
            if cc % 2 == 0:
                nc.vector.tensor_copy(ut_b[:, cc, :], ut_ps[i])
            else:
                nc.scalar.copy(ut_b[:, cc, :], ut_ps[i])

        for i, cc in enumerate(s1ccs):
            convert_ut(i, cc)

        for cc in range(4, DC):
            t = ps_ut.tile([P, R], F32, tag="ut", name=f"ut_ps{cc}")
            nc.vector.memset(t, 0.0)
            for kb in range(NKB):
                nc.tensor.matmul(
                    t,
                    xn_sb[:, kb, cc * P:(cc + 1) * P],
                    e_sb[:, kb, :],
                    start=False, stop=(kb == NKB - 1),
                    skip_group_check=True,
                )
            ut_ps.append(t)
            convert_ut(4 + (cc - 4), cc)

        # ---- out[q, dv] = (sum_c Ut[c,q] WvT[c,dv]) * recip[q] ----
        with tc.tile_pool(name="outp", bufs=2) as outp:
            for cq in range(QC):
                ot = outp.tile([P, D], BF16, tag="out")
                for nd in range(2):
                    ps = ps_mm.tile([P, R], F32, tag="mm")
                    for cc in range(DC):
                        nc.tensor.matmul(
                            ps,
                            ut_b[:, cc, cq * P:(cq + 1) * P],
                            wvt_sb[:, cc, nd * R:(nd + 1) * R],
                            start=(cc == 0), stop=(cc == DC - 1),
                        )
                    nc.vector.tensor_scalar_mul(
                        ot[:, nd * R:(nd + 1) * R], ps, recip[:, cq:cq + 1])
                nc.sync.dma_start(out=out_d[cq * P:(cq + 1) * P, :], in_=ot)


_CACHE = {}


def _get_program():
    if "nc" not in _CACHE:
        _CACHE["nc"] = build_program()
    return _CACHE["nc"]


def make_in_maps(x, W_query, W_key, W_value):
    x = np.ascontiguousarray(x, dtype=np.float32)
    wq = np.asarray(W_query, dtype=np.float32)
    wk = np.asarray(W_key, dtype=np.float32)
    wv = np.asarray(W_value, dtype=np.float32)
    at_f = wq.T @ wk                                      # A.T = lhsT of Wk.T @ Wq
    # at rows (bb, p) hold all jc-blocks contiguously: 2 KB DMA rows
    at = np.ascontiguousarray(
        at_f.reshape(DC, P, DC, P).transpose(2, 1, 0, 3).reshape(D, D)).astype(BF)
    xt = np.ascontiguousarray(x.T)
    # xt rows (sc, bp, p) hold the (i, k) pair-block contiguously
    xtb = np.ascontiguousarray(
        xt.reshape(DC // 2, 2, P, NSC, KSC).transpose(3, 0, 2, 1, 4)
        .reshape(NSC * (DC // 2) * P, 2 * KSC)).astype(BF)
    xb = x.astype(BF)
    wvt = np.ascontiguousarray(wv.T).astype(BF)
    maps = []
    for i in range(NCORES):
        xqt_f = xt[:, i * R:(i + 1) * R]
        xqt = np.ascontiguousarray(
            xqt_f.reshape(DC // 2, 2, P, R).transpose(0, 2, 1, 3)
            .reshape(D // 2, 2 * R)).astype(BF)
        maps.append({"at": at, "xqt": xqt, "xtb": xtb, "xb": xb, "wvt": wvt})
    return maps


def kernel(x, W_query, W_key, W_value):
    nc = _get_program()
    in_maps = make_in_maps(x, W_query, W_key, W_value)
    res = run_bass_kernel_spmd(nc, in_maps, core_ids=list(range(NCORES)))
    return np.concatenate(
        [res.results[i]["out"] for i in range(NCORES)], axis=0
    ).astype(np.float32)
